# revision 2
# baseline (speedup 1.0000x reference)
"""Trainium2 Bass kernel for nn_Model_39676907885209.

Per (batch, channel) pair: two 1x1 convs (spatial pad 1) produce keys/values
[512,512]; scores = K @ V^T / 0.12 -> softmax -> out = attn @ V.

Design (8 NeuronCores, data-parallel over batch, 2 batches/core):
 - Host marshals x1 into a padded, spatially-transposed, channel-interleaved
   layout (and a hi/lo bf16 split) so the 1x1 conv runs on the TensorEngine
   as col-group-packed matmuls with block-diagonal delta weights.
 - conv: 3 accumulating bf16 matmul rounds (w_hi*x_hi + w_hi*x_lo + w_lo*x_hi)
   -> fp32-class conv output in PSUM.
 - scores matmul in float32r (11-bit mantissa, full PE rate), with 1/0.12
   folded into the K-side conv weights.
 - softmax per 128-row tile: DVE reduce_max -> ACT Exp(bias=-max) with fused
   row-sum -> DVE reciprocal + scale; attn stored bf16.
 - attn and V(bf16) transposed via DMA-transpose (XBAR); AV matmul in bf16.
"""
import sys
sys.path.insert(0, '/opt/trn_rl_repo')

import numpy as np
import ml_dtypes

bf = ml_dtypes.bfloat16

INV = 1.0 / 0.12
N_CORES = 8
B_PER_CORE = 2
N_CH = 8

_cache = {}


def _build_program():
    import concourse.bacc as bacc
    import concourse.mybir as mybir
    from concourse import tile

    F32 = mybir.dt.float32
    F32R = mybir.dt.float32r
    BF16 = mybir.dt.bfloat16
    AL = mybir.AluOpType
    AFT = mybir.ActivationFunctionType

    nc = bacc.Bacc(None, target_bir_lowering=False)
    d_xh = nc.declare_dram_parameter("xh", [B_PER_CORE, 16, 96, 512], BF16, isOutput=False)
    d_xl = nc.declare_dram_parameter("xl", [B_PER_CORE, 16, 96, 512], BF16, isOutput=False)
    # weight delta-patterns: o 0..7 = K-conv (INV folded), 8..15 = V-conv
    d_wh = nc.declare_dram_parameter("wh", [16, 96, 32], BF16, isOutput=False)
    d_wl = nc.declare_dram_parameter("wl", [16, 96, 32], BF16, isOutput=False)
    d_bias = nc.declare_dram_parameter("bias", [128, 16], F32, isOutput=False)
    d_out = nc.declare_dram_parameter("out", [B_PER_CORE, N_CH, 512, 512], F32, isOutput=True)

    with tile.TileContext(nc) as tc:
        with tc.tile_pool(name="xin", bufs=2) as xin_pool, \
             tc.tile_pool(name="w", bufs=1) as w_pool, \
             tc.tile_pool(name="conv", bufs=2) as conv_pool, \
             tc.tile_pool(name="sm", bufs=3) as sm_pool, \
             tc.tile_pool(name="att", bufs=2) as att_pool, \
             tc.tile_pool(name="outp", bufs=3) as out_pool, \
             tc.tile_pool(name="psc", bufs=2, space="PSUM") as psc, \
             tc.tile_pool(name="pss", bufs=2, space="PSUM") as pss, \
             tc.tile_pool(name="pso", bufs=2, space="PSUM") as pso:

            whs, wls = [], []
            for o in range(16):
                wh_t = w_pool.tile([96, 32], BF16, tag=f"wh{o}")
                nc.gpsimd.dma_start(wh_t[:], d_wh[o])
                whs.append(wh_t)
                wl_t = w_pool.tile([96, 32], BF16, tag=f"wl{o}")
                nc.gpsimd.dma_start(wl_t[:], d_wl[o])
                wls.append(wl_t)
            bias_t = w_pool.tile([128, 16], F32, tag="bias")
            nc.gpsimd.dma_start(bias_t[:], d_bias[:])

            for b in range(B_PER_CORE):
                xhs, xls = [], []
                for wc in range(16):
                    th = xin_pool.tile([96, 512], BF16, tag=f"xh{wc}")
                    nc.gpsimd.dma_start(th[:], d_xh[b, wc])
                    xhs.append(th)
                    tl = xin_pool.tile([96, 512], BF16, tag=f"xl{wc}")
                    nc.gpsimd.dma_start(tl[:], d_xl[b, wc])
                    xls.append(tl)

                for o in range(N_CH):
                    # ---- conv: X2T (keys^T, f32r), X3T (values^T, f32r + bf16) ----
                    def conv_plane(wh_t, wl_t, wt):
                        p = psc.tile([128, 512], F32, tag="pconv")
                        for j in range(4):
                            sl = p[32 * j:32 * (j + 1), :]
                            xh_ap = xhs[wt * 4 + j][:]
                            xl_ap = xls[wt * 4 + j][:]
                            tp = (0, 32 * j)
                            nc.tensor.matmul(sl, wh_t[:], xh_ap, start=True, stop=False, tile_position=tp)
                            nc.tensor.matmul(sl, wh_t[:], xl_ap, start=False, stop=False, tile_position=tp)
                            nc.tensor.matmul(sl, wl_t[:], xh_ap, start=False, stop=True, tile_position=tp)
                        return p

                    x2t, x3t, x3tb = [], [], []
                    for wt in range(4):
                        pk = conv_plane(whs[o], wls[o], wt)
                        t2 = conv_pool.tile([128, 512], F32R, tag=f"x2t{wt}")
                        nc.vector.tensor_scalar(t2[:], pk[:], bias_t[:, o:o + 1], None, AL.add)
                        x2t.append(t2)
                        pv = conv_plane(whs[8 + o], wls[8 + o], wt)
                        t3 = conv_pool.tile([128, 512], F32R, tag=f"x3t{wt}")
                        nc.scalar.activation(t3[:], pv[:], AFT.Identity, bias=bias_t[:, 8 + o:9 + o], scale=1.0)
                        x3t.append(t3)
                        t3b = conv_pool.tile([128, 512], BF16, tag=f"x3tb{wt}")
                        nc.scalar.activation(t3b[:], pv[:], AFT.Identity, bias=bias_t[:, 8 + o:9 + o], scale=1.0)
                        x3tb.append(t3b)

                    x3n = []
                    for kt in range(4):
                        x3n_t = conv_pool.tile([128, 512], BF16, tag=f"x3n{kt}")
                        x3n.append(x3n_t)
                    for kt in range(4):
                        for wt in range(4):
                            nc.sync.dma_start_transpose(
                                x3n[kt][:, 128 * wt:128 * (wt + 1)],
                                x3tb[wt][:, 128 * kt:128 * (kt + 1)])

                    # ---- scores (f32r) + softmax + attn transpose ----
                    attnT = []
                    for kt in range(4):
                        attnT_t = att_pool.tile([128, 512], BF16, tag=f"attnT{kt}")
                        attnT.append(attnT_t)
                    for m in range(4):
                        ps = pss.tile([128, 512], F32, tag="scores")
                        for wt in range(4):
                            nc.tensor.matmul(ps[:], x2t[wt][:, 128 * m:128 * (m + 1)], x3t[wt][:],
                                             start=(wt == 0), stop=(wt == 3))
                        negmax = sm_pool.tile([128, 1], F32, tag="negmax")
                        nc.vector.tensor_reduce(negmax[:], ps[:], mybir.AxisListType.X, AL.max, negate=True)
                        esum = sm_pool.tile([128, 1], F32, tag="esum")
                        E = sm_pool.tile([128, 512], BF16, tag="E")
                        nc.scalar.activation(E[:], ps[:], AFT.Exp, bias=negmax[:], scale=1.0, accum_out=esum[:])
                        rec = sm_pool.tile([128, 1], F32, tag="rec")
                        nc.vector.reciprocal(rec[:], esum[:])
                        A = sm_pool.tile([128, 512], BF16, tag="A")
                        nc.vector.tensor_scalar(A[:], E[:], rec[:], None, AL.mult)
                        for kt in range(4):
                            nc.sync.dma_start_transpose(
                                attnT[kt][:, 128 * m:128 * (m + 1)],
                                A[:, 128 * kt:128 * (kt + 1)])

                    # ---- out = attn @ V (bf16) ----
                    for m in range(4):
                        po = pso.tile([128, 512], F32, tag="outps")
                        for kt in range(4):
                            nc.tensor.matmul(po[:], attnT[kt][:, 128 * m:128 * (m + 1)], x3n[kt][:],
                                             start=(kt == 0), stop=(kt == 3))
                        so = out_pool.tile([128, 512], F32, tag="so")
                        nc.scalar.copy(so[:], po[:])
                        nc.sync.dma_start(d_out[b, o, 128 * m:128 * (m + 1), :], so[:])

    nc.compile()
    return nc


def _host_prep(x1, Wk, bk, Wv, bv):
    """Marshal inputs into the device layouts. Returns per-core in_maps."""
    B = x1.shape[0]
    xp = np.pad(x1, ((0, 0), (0, 0), (1, 1), (1, 1)))       # [B,3,512,512] (h,w)
    xpT = xp.transpose(0, 1, 3, 2)                           # [B,3,512,512] (w,h)
    # [B, wc=16, p=96, h=512] with p = c*32 + wj, w = wc*32 + wj
    xint = np.ascontiguousarray(
        xpT.reshape(B, 3, 16, 32, 512).transpose(0, 2, 1, 3, 4)).reshape(B, 16, 96, 512)
    xh = xint.astype(bf)
    xl = (xint - xh.astype(np.float32)).astype(bf)

    wk_s = (Wk.astype(np.float64) * INV).astype(np.float32)  # [8,3]
    wv_s = Wv.astype(np.float32)
    w_all = np.concatenate([wk_s, wv_s], axis=0)             # [16,3]
    w_h = w_all.astype(bf).astype(np.float32)
    w_l = w_all - w_h
    eye = np.eye(32, dtype=np.float32)
    # [16, 96, 32]
    Wp_h = np.zeros((16, 96, 32), dtype=np.float32)
    Wp_l = np.zeros((16, 96, 32), dtype=np.float32)
    for c in range(3):
        Wp_h[:, c * 32:(c + 1) * 32, :] = eye[None] * w_h[:, c][:, None, None]
        Wp_l[:, c * 32:(c + 1) * 32, :] = eye[None] * w_l[:, c][:, None, None]
    Wp_h = Wp_h.astype(bf)
    Wp_l = Wp_l.astype(bf)

    bias = np.zeros((128, 16), dtype=np.float32)
    bias[:, :8] = (bk.astype(np.float64) * INV).astype(np.float32)[None, :]
    bias[:, 8:] = bv.astype(np.float32)[None, :]

    in_maps = []
    for c in range(N_CORES):
        sl = slice(c * B_PER_CORE, (c + 1) * B_PER_CORE)
        in_maps.append({
            "xh": np.ascontiguousarray(xh[sl]),
            "xl": np.ascontiguousarray(xl[sl]),
            "wh": Wp_h, "wl": Wp_l, "bias": bias,
        })
    return in_maps


def kernel(x1, Wk, bk, Wv, bv):
    from concourse.bass_utils import run_bass_kernel_spmd

    if "nc" not in _cache:
        _cache["nc"] = _build_program()
    nc = _cache["nc"]

    in_maps = _host_prep(np.asarray(x1, dtype=np.float32),
                         np.asarray(Wk, dtype=np.float32),
                         np.asarray(bk, dtype=np.float32),
                         np.asarray(Wv, dtype=np.float32),
                         np.asarray(bv, dtype=np.float32))
    res = run_bass_kernel_spmd(nc, in_maps, list(range(N_CORES)))
    out = np.concatenate([res.results[c]["out"] for c in range(N_CORES)], axis=0)
    return out.astype(np.float32)
